# revision 1
# baseline (speedup 1.0000x reference)
"""Bass kernel builder for DeformConv2d — one sample per NeuronCore.

Per-core pipeline:
  Stage 1 (H/8 strips x 8 rows):
    sync: x strip -> xs [64, 10, 514] f32 (halo rows, zero side cols)
    PE:   offset conv (9 f32 MMs/row, PSUM acc); uT tap images per 128-px chunk
          (x-chunk stationary [64,128], wd_all moving [64,576], f32);
          offT transposes (16-chunk batches)
    DVE:  off psum + b_off -> off_sb (4-row ring); offT psum -> staging
    ACT:  cast uT psum -> ustage bf16 (2-row ring); u2 border zero fills
    sync: double-store ustage -> u2 [9, H+8, 520, 128] bf16; offT staging -> offT_d
  Stage 2 (H/16 substrips x 16 rows):
    sync: offT_d -> offT2 (dbl buffered); acc -> out
    DVE:  frac/floor/weights/idx (pixel-partition, batched over taps);
          acc = b_def + sum of 36 weighted gathered corners
    ACT:  idx fold DMAs -> wrapped gather layout
    GPSIMD: dma_gather 256B pixel-pair blocks from u2 windows -> G ring
"""
import numpy as np
from contextlib import ExitStack

import concourse.bass as bass
import concourse.bacc as bacc
import concourse.mybir as mybir
from concourse import library_config

F32 = mybir.dt.float32
BF16 = mybir.dt.bfloat16
I16 = mybir.dt.int16
OP = mybir.AluOpType
ACTF = mybir.ActivationFunctionType

C = 64
CO = 64
W = 512
WP = 520
PAD = 4
NT = 9


def build(H=128, do_gather=True, do_mac=True):
    assert H % 16 == 0
    HP = H + 2 * PAD
    NS1 = H // 8
    NS2 = H // 16
    NB = NS1 * 2     # offT batches (16 chunks)
    NE = NS1 * 4     # eighths (2 rows)

    nc = bacc.Bacc("TRN2")

    x_in = nc.declare_dram_parameter("x", [C, H, W], F32, isOutput=False)
    w_offT_in = nc.declare_dram_parameter("w_offT", [C, NT, 18], F32, isOutput=False)
    b_off_in = nc.declare_dram_parameter("b_off", [18, 1], F32, isOutput=False)
    wd_in = nc.declare_dram_parameter("wd_all", [C, NT * CO], F32, isOutput=False)
    bdef_in = nc.declare_dram_parameter("bdef", [128, CO], F32, isOutput=False)
    ident_in = nc.declare_dram_parameter("ident", [128, 128], F32, isOutput=False)
    base_in = nc.declare_dram_parameter("base_idx", [128, 64], F32, isOutput=False)
    ct_in = nc.declare_dram_parameter("const_t", [128, NT], F32, isOutput=False)
    bxc_in = nc.declare_dram_parameter("bxc", [128, 64, NT], F32, isOutput=False)
    out_d = nc.declare_dram_parameter("out", [H * W, CO], F32, isOutput=True)

    u2 = nc.dram_tensor("u2", [NT, HP, W, 128], BF16)
    ub = nc.dram_tensor("ub", [128, H, 4, NT, CO], BF16)
    offT_d = nc.dram_tensor("offT_d", [NB, 128, 16 * 18], F32)

    es = ExitStack()
    _uid = [0]

    def sb(shape, dt):
        _uid[0] += 1
        return es.enter_context(nc.sbuf_tensor(f"sb{_uid[0]}", shape, dt))

    def psum(shape):
        _uid[0] += 1
        return es.enter_context(nc.psum_tensor(f"ps{_uid[0]}", shape, F32))

    def sem(name):
        return es.enter_context(nc.semaphore(name))

    xs = sb([C, 10, 514], F32)
    w_off_sb = sb([C, NT, 18], F32)
    b_off_sb = sb([18, 1], F32)
    wd_sb = sb([C, NT * CO], F32)
    bdef_sb = sb([128, CO], F32)
    ident_sb = sb([128, 128], F32)
    base_sb = sb([128, 64], F32)
    ct_sb = sb([128, NT], F32)
    bxc_sb = sb([128, 64, NT], F32)
    xpos = sb([128, 64, NT], F32)
    vx = sb([128, 64, NT], F32)
    off_sb = sb([18, 4, W], F32)
    oT_stg = sb([128, 2, 16 * 18], F32)
    ustage = sb([128, 2, 8, NT * CO], BF16)
    zeros_sb = sb([128, 2048], BF16)
    offT2 = sb([128, 2, 64, 18], F32)
    fy = sb([128, 64, NT], F32)
    fx = sb([128, 64, NT], F32)
    gy = sb([128, 64, NT], F32)
    gx = sb([128, 64, NT], F32)
    wy0 = sb([128, 64, NT], F32)
    wx0 = sb([128, 64, NT], F32)
    ti32 = sb([128, 64, NT], mybir.dt.int32)
    wgt = sb([128, 64, NT, 4], BF16)
    idxf = sb([128, 64, NT, 2], F32)
    idxs = sb([128, NT * 2, 64], I16)
    wrapped = sb([128, NT * 2, 512], I16)
    G = [sb([128, 32, 128], BF16) for _ in range(4)]
    mtmp = sb([128, 32, CO], F32)
    acc = sb([128, 64, CO], F32)

    off_ps = [psum([18, 512]) for _ in range(2)]
    uT_ps8 = [psum([128, 512]) for _ in range(2)]
    uT_ps1 = [psum([128, 512]) for _ in range(2)]   # only [:, 0:64] used; bank pad
    offT_ps = [psum([128, 512]) for _ in range(2)]  # only [:, 0:288] used

    s_in = sem("s_in"); s_ms = sem("s_ms"); s_x = sem("s_x"); s_u = sem("s_u")
    s_z = sem("s_z"); s_pe = sem("s_pe"); s_pu = sem("s_pu"); s_pt = sem("s_pt")
    s_vo = sem("s_vo"); s_vt = sem("s_vt"); s_ot = sem("s_ot"); s_o2 = sem("s_o2")
    s_ac = sem("s_ac"); s_w = sem("s_w"); s_if = sem("s_if"); s_g = sem("s_g")
    s_m = sem("s_m"); s_o = sem("s_o"); s_u2 = sem("s_u2"); s_gb2 = sem("s_gb2")

    N_IN = 8
    FOLDS = 15
    z_total = NT * (3 + (1 if HP > 128 else 0))

    zf32 = zeros_sb[:].bitcast(F32)   # [128, 1040] f32 zero view
    h128 = min(HP, 128)               # side-border partition split

    def sx_total(k):
        """s_x value after strip-k loads complete."""
        return 16 * (k + 1) + (16 if k + 1 == NS1 else 0)

    es2 = ExitStack()
    with nc.Block() as block:

        # ================= SYNC =================
        @block.sync
        def _(e):
            e.dma_start(w_off_sb[:], w_offT_in[:]).then_inc(s_in, 16)
            e.dma_start(b_off_sb[:], b_off_in[:]).then_inc(s_in, 16)
            e.dma_start(wd_sb[:], wd_in[:]).then_inc(s_in, 16)
            e.dma_start(bdef_sb[:], bdef_in[:]).then_inc(s_in, 16)
            e.dma_start(ident_sb[:], ident_in[:]).then_inc(s_in, 16)
            e.dma_start(base_sb[:], base_in[:]).then_inc(s_in, 16)
            e.dma_start(ct_sb[:], ct_in[:]).then_inc(s_in, 16)
            e.dma_start(bxc_sb[:], bxc_in[:]).then_inc(s_in, 16)

            e.wait_ge(s_ms, 1)
            for k in range(NS1):
                r0 = 8 * k - 1
                lo, hi = max(r0, 0), min(r0 + 10, H)
                e.dma_start(xs[:, lo - r0:hi - r0, 1:513], x_in[:, lo:hi, :]
                            ).then_inc(s_x, 16)
                if hi - r0 < 10:
                    e.dma_start(xs[:, hi - r0, 1:513], zf32[0:C, 0:512]
                                ).then_inc(s_x, 16)
                for ee in range(4):
                    q = 4 * k + ee
                    e.wait_ge(s_ac, q * 8 + 8)
                    dst = ub[:, 8 * k + 2 * ee:8 * k + 2 * ee + 2, :, :, :] \
                        .rearrange("p y b t c -> p (y b) (t c)")
                    e.dma_start(dst, ustage[:, q % 2, :, :]).then_inc(s_u, 16)
                for bt in range(2):
                    gb = 2 * k + bt
                    e.wait_ge(s_vt, gb + 1)
                    e.dma_start(offT_d[gb, :, :], oT_stg[:, gb % 2, :]).then_inc(s_ot, 16)
                if k + 1 < NS1:
                    e.wait_ge(s_pu, 32 * (k + 1))

            # u2 assembly: 18 whole-image D2D rearrange DMAs (lo+hi halves)
            e.wait_ge(s_u, NE * 16)
            e.wait_ge(s_z, z_total * 16)
            u2f = u2.rearrange("t y x c -> t (y x) c")
            H2 = H // 2
            for t in range(NT):
                for hf in range(2):
                    srcu = ub[:, hf * H2:(hf + 1) * H2, :, t, :] \
                        .rearrange("p y b c -> p (y b) c")
                    dlo = u2[t, PAD + hf * H2:PAD + (hf + 1) * H2, :, 0:64] \
                        .rearrange("y (b p) c -> p (y b) c", p=128)
                    e.dma_start(dlo, srcu).then_inc(s_u2, 16)
                    r0 = (PAD + hf * H2) * W - 1
                    dhi = u2f[t, r0:r0 + H2 * W, 64:128] \
                        .rearrange("(yb p) c -> p yb c", p=128)
                    e.dma_start(dhi, srcu).then_inc(s_u2, 16)

            # stage 2: interleave reloads and out-writes (deadlock-free order)
            def reload(s):
                if s >= 2:
                    e.wait_ge(s_w, s - 1)
                e.wait_ge(s_ot, NB * 16)
                src = offT_d[4 * s:4 * s + 4, :, :].rearrange("b p f -> p b f")
                dst = offT2[:, s % 2, :, :].rearrange("p c q -> p (c q)") \
                    .rearrange("p (b f) -> p b f", b=4)
                e.dma_start(dst, src).then_inc(s_o2, 16)

            reload(0)
            if NS2 > 1:
                reload(1)
            for s in range(NS2):
                e.wait_ge(s_m, 18 * (s + 1))
                dst = out_d[s * 16 * W:(s + 1) * 16 * W, :] \
                    .rearrange("(y b p) c -> p y b c", b=4, p=128)
                e.dma_start(dst, acc[:].rearrange("p (y b) c -> p y b c", b=4)
                            ).then_inc(s_o, 16)
                if s + 2 < NS2:
                    reload(s + 2)
            e.wait_ge(s_o, NS2 * 16)

        # ================= ACT =================
        @block.scalar
        def _(e):
            e.wait_ge(s_ms, 1)
            for t in range(NT):
                top = u2[t, 0:PAD, :, :].rearrange("y x c -> (y x c)") \
                    .rearrange("(p f) -> p f", p=128)
                e.dma_start(top, zeros_sb[:, 0:2048]).then_inc(s_z, 16)
                bot = u2[t, HP - PAD:HP, :, :].rearrange("y x c -> (y x c)") \
                    .rearrange("(p f) -> p f", p=128)
                e.dma_start(bot, zeros_sb[:, 0:2048]).then_inc(s_z, 16)
                c511 = u2[t, :, W - 1, 64:128]
                e.dma_start(c511[0:h128, :], zeros_sb[0:h128, 0:64]).then_inc(s_z, 16)
                if HP > 128:
                    e.dma_start(c511[128:HP, :], zeros_sb[0:HP - 128, 0:64]
                                ).then_inc(s_z, 16)

            for k in range(NS1):
                for lc in range(32):
                    g = 32 * k + lc
                    q, slot = g // 8, g % 8
                    if slot == 0 and q >= 2:
                        e.wait_ge(s_u, q * 16)
                    e.wait_ge(s_pu, g + 1)
                    pb = g % 2
                    nc.scalar.activation(ustage[:, q % 2, slot, 0:512],
                                         uT_ps8[pb][:], ACTF.Copy)
                    nc.scalar.activation(ustage[:, q % 2, slot, 512:576],
                                         uT_ps1[pb][:, 0:64], ACTF.Copy
                                         ).then_inc(s_ac, 1)

            for s in range(NS2):
                e.wait_ge(s_w, s + 1)
                if s >= 1:
                    e.wait_ge(s_g, 9 * s * 32)
                    e.wait_ge(s_gb2, 9 * s * 32)
                with nc.allow_non_contiguous_dma(reason="tiny idx fold"):
                    for q in range(8):
                        e.dma_start(wrapped[0:16, :, q::8],
                                    idxs[16 * q:16 * q + 16, :, :]).then_inc(s_if, 16)
                e.wait_ge(s_if, s * FOLDS * 16 + 8 * 16)
                for kk in range(1, 8):
                    e.dma_start(wrapped[16 * kk:16 * kk + 16, :, :], wrapped[0:16, :, :]
                                ).then_inc(s_if, 16)

        # ================= PE =================
        @block.tensor
        def _(e):
            e.wait_ge(s_in, N_IN * 16)
            for k in range(NS1):
                e.wait_ge(s_x, sx_total(k))

                def off_row(r):
                    gr = 8 * k + r
                    pb = gr % 2
                    if gr >= 2:
                        e.wait_ge(s_vo, gr - 1)
                    mm = None
                    for t in range(NT):
                        i, j = t // 3, t % 3
                        mm = nc.tensor.matmul(off_ps[pb][:], w_off_sb[:, t, :],
                                              xs[:, r + i, j:j + 512],
                                              start=(t == 0), stop=(t == NT - 1))
                    mm.then_inc(s_pe, 1)

                def tr_batch(bt):
                    gb = 2 * k + bt
                    pb = gb % 2
                    e.wait_ge(s_vo, 8 * k + 4 * (bt + 1))
                    if gb >= 2:
                        e.wait_ge(s_vt, gb - 1)
                    mm = None
                    for c16 in range(16):
                        lc = bt * 16 + c16
                        rr, b = (lc // 4) % 4, lc % 4
                        mm = nc.tensor.transpose(
                            offT_ps[pb][:, c16 * 18:(c16 + 1) * 18],
                            off_sb[:, rr, b * 128:(b + 1) * 128],
                            ident_sb[0:18, 0:18])
                    mm.then_inc(s_pt, 1)

                for r in range(4):
                    off_row(r)
                tr_batch(0)
                for r in range(4, 8):
                    off_row(r)
                tr_batch(1)
                for lc in range(32):
                    g = 32 * k + lc
                    pb = g % 2
                    r, b = lc // 4, lc % 4
                    if g >= 2:
                        e.wait_ge(s_ac, g - 1)
                    lhsT = xs[:, r + 1, 1 + b * 128:1 + b * 128 + 128]
                    nc.tensor.matmul(uT_ps8[pb][:], lhsT, wd_sb[:, 0:512],
                                     start=True, stop=True)
                    nc.tensor.matmul(uT_ps1[pb][:, 0:64], lhsT, wd_sb[:, 512:576],
                                     start=True, stop=True).then_inc(s_pu, 1)

        # ================= DVE =================
        @block.vector
        def _(e):
            nc.vector.memset(xs[:], 0.0)
            nc.vector.memset(zeros_sb[:], 0.0)
            e.drain()
            e.sem_inc(s_ms, 1)
            e.wait_ge(s_in, N_IN * 16)
            for k in range(NS1):
                for r in range(8):
                    gr = 8 * k + r
                    pb = gr % 2
                    e.wait_ge(s_pe, gr + 1)
                    if gr >= 4:
                        e.wait_ge(s_pt, (gr - 4) // 4 + 1)
                    nc.vector.tensor_scalar(off_sb[:, r % 4, :], off_ps[pb][:],
                                            b_off_sb[:], 0.0, OP.add, OP.add
                                            ).then_inc(s_vo, 1)
                for bt in range(2):
                    gb = 2 * k + bt
                    pb = gb % 2
                    e.wait_ge(s_pt, gb + 1)
                    if gb >= 2:
                        e.wait_ge(s_ot, gb * 16)
                    nc.vector.tensor_copy(oT_stg[:, gb % 2, :], offT_ps[pb][:, 0:288]
                                          ).then_inc(s_vt, 1)

            for s in range(NS2):
                e.wait_ge(s_o2, 16 * min(s + 2, NS2))
                oT = offT2[:, s % 2, :, :]
                dy = oT[:, :, 0:18:2]
                dx = oT[:, :, 1:18:2]
                # floor via int cast: t = int(v); floor = t - (t > v)
                nc.vector.tensor_copy(ti32[:], dy)
                e.drain()
                nc.vector.tensor_copy(fy[:], ti32[:])
                e.drain()
                nc.vector.tensor_tensor(vx[:], fy[:], dy, OP.is_gt)
                e.drain()
                nc.vector.tensor_tensor(fy[:], fy[:], vx[:], OP.subtract)
                e.drain()
                nc.vector.tensor_copy(ti32[:], dx)
                e.drain()
                nc.vector.tensor_copy(fx[:], ti32[:])
                e.drain()
                nc.vector.tensor_tensor(vx[:], fx[:], dx, OP.is_gt)
                e.drain()
                nc.vector.tensor_tensor(fx[:], fx[:], vx[:], OP.subtract)
                e.drain()
                nc.vector.tensor_tensor(gy[:], dy, fy[:], OP.subtract)
                nc.vector.tensor_tensor(gx[:], dx, fx[:], OP.subtract)
                e.drain()
                nc.vector.tensor_scalar(fy[:], fy[:], -3.0, 2.0, OP.max, OP.min)
                nc.vector.tensor_scalar(fx[:], fx[:], -3.0, 2.0, OP.max, OP.min)
                nc.vector.tensor_scalar(wy0[:], gy[:], -1.0, 1.0, OP.mult, OP.add)
                nc.vector.tensor_scalar(wx0[:], gx[:], -1.0, 1.0, OP.mult, OP.add)
                e.drain()
                # x-validity: corner x0 = fx + bxc in [0, 511]; x1 = x0 + 1
                nc.vector.tensor_tensor(xpos[:], fx[:], bxc_sb[:], OP.add)
                e.drain()
                # vx0 = (xpos >= 0) * (xpos <= 511) ; wx0 *= vx0
                nc.vector.tensor_scalar(vx[:], xpos[:], 0.0, 1.0, OP.is_ge, OP.mult)
                e.drain()
                nc.vector.tensor_tensor(wx0[:], wx0[:], vx[:], OP.mult)
                e.drain()
                nc.vector.tensor_scalar(vx[:], xpos[:], 511.0, 1.0, OP.is_le, OP.mult)
                e.drain()
                nc.vector.tensor_tensor(wx0[:], wx0[:], vx[:], OP.mult)
                e.drain()
                # vx1 = (xpos >= -1) * (xpos <= 510) ; wx1(gx) *= vx1
                nc.vector.tensor_scalar(vx[:], xpos[:], -1.0, 1.0, OP.is_ge, OP.mult)
                e.drain()
                nc.vector.tensor_tensor(gx[:], gx[:], vx[:], OP.mult)
                e.drain()
                nc.vector.tensor_scalar(vx[:], xpos[:], 510.0, 1.0, OP.is_le, OP.mult)
                e.drain()
                nc.vector.tensor_tensor(gx[:], gx[:], vx[:], OP.mult)
                e.drain()
                nc.vector.tensor_tensor(wgt[:, :, :, 0], wy0[:], wx0[:], OP.mult)
                nc.vector.tensor_tensor(wgt[:, :, :, 1], wy0[:], gx[:], OP.mult)
                nc.vector.tensor_tensor(wgt[:, :, :, 2], gy[:], wx0[:], OP.mult)
                nc.vector.tensor_tensor(wgt[:, :, :, 3], gy[:], gx[:], OP.mult)
                nc.vector.tensor_scalar(idxf[:, :, :, 0], fy[:], 512.0, 0.0,
                                        OP.mult, OP.add)
                e.drain()
                nc.vector.tensor_tensor(idxf[:, :, :, 0], idxf[:, :, :, 0], fx[:], OP.add)
                e.drain()
                nc.vector.tensor_tensor(idxf[:, :, :, 0], idxf[:, :, :, 0],
                                        base_sb[:].unsqueeze(2)
                                        .broadcast_to([128, 64, NT]), OP.add)
                e.drain()
                nc.vector.tensor_tensor(idxf[:, :, :, 0], idxf[:, :, :, 0],
                                        ct_sb[:].unsqueeze(1)
                                        .broadcast_to([128, 64, NT]), OP.add)
                e.drain()
                nc.vector.tensor_scalar(idxf[:, :, :, 1], idxf[:, :, :, 0], 512.0, 0.0,
                                        OP.add, OP.add)
                e.drain()
                nc.vector.tensor_scalar(idxf[:], idxf[:], 0.0, 12287.0, OP.max, OP.min)
                e.drain()
                nc.vector.tensor_copy(
                    idxs[:].rearrange("p (t a) c -> p t a c", a=2)
                    .transpose([0, 3, 1, 2]), idxf[:])
                e.drain()
                e.sem_inc(s_w, 1)

                if s >= 1:
                    e.wait_ge(s_o, 16 * s)
                nc.vector.tensor_copy(acc[:], bdef_sb[:].unsqueeze(1)
                                      .broadcast_to([128, 64, CO]))
                e.drain()
                for t in range(NT):
                    for hh in range(2):
                        gseq = 36 * s + 4 * t + 2 * hh
                        gp = gseq // 2
                        e.wait_ge(s_g if gp % 2 == 0 else s_gb2,
                                  (gp // 2 + 1) * 32)
                        lo, hi = G[gseq % 4], G[(gseq + 1) % 4]
                        ah = acc[:, 32 * hh:32 * hh + 32, :]
                        if do_mac:
                            for gt, wi in ((lo, 0), (lo, 1), (hi, 2), (hi, 3)):
                                half = gt[:, :, 0:64] if wi % 2 == 0 else gt[:, :, 64:128]
                                wb = wgt[:, 32 * hh:32 * hh + 32, t, wi] \
                                    .unsqueeze(2).broadcast_to([128, 32, CO])
                                nc.vector.tensor_tensor(mtmp[:], half, wb, OP.mult)
                                e.drain()
                                nc.vector.tensor_tensor(ah, ah, mtmp[:], OP.add)
                                e.drain()
                        e.sem_inc(s_m, 1)

        # ================= GPSIMD =================
        @block.gpsimd
        def _(e):
            nidx_reg = es2.enter_context(e.register("nidx"))
            e.reg_mov(nidx_reg, 4096)
            e.wait_ge(s_u2, NT * 4 * 16)
            for s in range(NS2):
                e.wait_ge(s_if, (s + 1) * FOLDS * 16)
                for t in range(NT):
                    win = u2[t, 16 * s:16 * s + 24, :, :].rearrange("y x c -> (y x) c")
                    for hh in range(2):
                        for a in range(2):
                            gseq = 36 * s + 4 * t + 2 * hh + a
                            gp = gseq // 2
                            if gseq >= 4:
                                e.wait_ge(s_m, gp - 1)
                            if do_gather:
                                e.dma_gather(
                                    out_ap=G[gseq % 4][:],
                                    in_ap=win,
                                    idxs_ap=wrapped[:, 2 * t + a, 256 * hh:256 * hh + 256],
                                    num_idxs=4096,
                                    num_idxs_reg=nidx_reg,
                                    elem_size=128,
                                    elem_step=128,
                                    single_packet=False,
                                ).then_inc(s_g if gp % 2 == 0 else s_gb2, 16)
                            else:
                                e.sem_inc(s_g if gp % 2 == 0 else s_gb2, 16)

    es2.close()
    es.close()
    nc.compile()
    return nc


def host_inputs(x_n, w_off, b_off, w_def, b_def):
    """Per-core input map for one sample (np arrays as in setup_inputs)."""
    w_offT = np.ascontiguousarray(
        w_off.reshape(18, C, NT).transpose(1, 2, 0), np.float32)     # [c, t, o=18]
    wd = np.ascontiguousarray(
        w_def.reshape(CO, C, NT).transpose(1, 2, 0), np.float32)     # [c, t, o]
    wd = np.ascontiguousarray(wd.reshape(C, NT * CO))
    base = ((np.arange(64)[None, :] // 4) + 3) * 512 \
        + (np.arange(64)[None, :] % 4) * 128 + np.arange(128)[:, None]
    ct = np.zeros((128, NT), np.float32)
    for t in range(NT):
        ct[:, t] = (t // 3) * 512 + (t % 3) - 1
    wcoord = (np.arange(64)[None, :] % 4) * 128 + np.arange(128)[:, None]  # [128, 64]
    bxc = np.zeros((128, 64, NT), np.float32)
    for t in range(NT):
        bxc[:, :, t] = wcoord + (t % 3) - 1
    return {
        "x": np.ascontiguousarray(x_n, np.float32),
        "w_offT": w_offT,
        "b_off": np.ascontiguousarray(b_off.reshape(18, 1), np.float32),
        "wd_all": wd,
        "bdef": np.ascontiguousarray(np.tile(b_def[None, :], (128, 1)), np.float32),
        "ident": np.eye(128, dtype=np.float32),
        "base_idx": base.astype(np.float32),
        "const_t": ct,
        "bxc": bxc,
    }


_CACHE = {}


def _get_nc():
    if "nc" not in _CACHE:
        _CACHE["nc"] = build(H=128)
    return _CACHE["nc"]


def kernel(x, w_off, b_off, w_def, b_def):
    """Full-input DeformConv2d on 8 NeuronCores (one sample per core)."""
    from concourse.bass_utils import run_bass_kernel_spmd

    x = np.asarray(x, np.float32)
    w_off = np.asarray(w_off, np.float32)
    b_off = np.asarray(b_off, np.float32)
    w_def = np.asarray(w_def, np.float32)
    b_def = np.asarray(b_def, np.float32)
    N, Cc, H, Wc = x.shape
    assert (N, Cc, H, Wc) == (8, 64, 128, 512)

    nc = _get_nc()
    shared = host_inputs(x[0], w_off, b_off, w_def, b_def)
    in_maps = []
    for n in range(N):
        m = dict(shared)
        m["x"] = np.ascontiguousarray(x[n], np.float32)
        in_maps.append(m)
    res = run_bass_kernel_spmd(nc, in_maps, list(range(N)))
    out = np.empty((N, CO, H, Wc), np.float32)
    for n in range(N):
        o = res.results[n]["out"]          # [H*W, CO]
        out[n] = o.reshape(H, Wc, CO).transpose(2, 0, 1)
    return out



# revision 20
# speedup vs baseline: 1.3747x; 1.3747x over previous
"""Bass kernel for DeformConv2d — one sample per NeuronCore, no gathers.

Math: bilinear at base+d equals sum_r tri(d - r) * u[base + r] over integer
r, where tri(a) = max(0, 1-|a|). With |d| <= 1 the support is r in
{-1,0,1}; tails |d| in (1, 2) are handled exactly by two extra corners
(r = +-2) with weights (d-1)+ / (-d-1)+ folded into extended main weights.
Per tap: 9 main + 12 rare corner terms, all regular shifted reads of the
tap image u_t = wd_t^T x — data-dependent gathers are eliminated.

Layout: stage-2 runs x-block partitioned (partition = x//4), pixels in the
free dims with px LAST (packed bf16) so DVE hits 2x/4x perf modes; channel
dim sits mid-AP where a stride-0 broadcast is free.

Per-core pipeline (strips of 8 rows, stage 2 lags stage 1 by 2 strips):
  stage 1: PE bf16 off-conv + uT matmuls; PE transposes off into x-block
           layout; ACT copies uT psum -> bf16 row staging; sync DMAs rows
           into u2 [9, 134, 520, 64] (y-pad 3, x-pad 4, zero borders).
  stage 2: sync loads off tile + 9 u tiles (ch-last, 1KB descriptors);
           ACT permutes tiles to px-last; DVE computes tri weights and the
           21-corner MAC per tap (bf16 2x mults + 4x scalar_tensor_tensor
           adds into a bf16 accumulator); Pool flushes each tap into the
           f32 acc; ACT permutes acc; sync stores to out [y, x, ch].
"""
import numpy as np
from contextlib import ExitStack

import concourse.bass as bass
import concourse.bacc as bacc
import concourse.mybir as mybir

F32 = mybir.dt.float32
BF16 = mybir.dt.bfloat16
OP = mybir.AluOpType
ACTF = mybir.ActivationFunctionType

C = 64
CO = 64
W = 512
NT = 9


def build(H=128):
    assert H % 8 == 0
    NS = H // 8                      # 8-row strips
    HP = H + 6                       # u2 y-pad 3+3
    WPU = W + 8                      # u2 x-pad 4+4

    nc = bacc.Bacc("TRN2")

    x_in = nc.declare_dram_parameter("x", [C, H, W], BF16, isOutput=False)
    w_offT_in = nc.declare_dram_parameter("w_offT", [C, NT, 18], BF16, isOutput=False)
    b_off_in = nc.declare_dram_parameter("b_off", [18, 1], F32, isOutput=False)
    wd_in = nc.declare_dram_parameter("wd_all", [C, NT * CO], BF16, isOutput=False)
    bdef_in = nc.declare_dram_parameter("bdef", [128, CO], F32, isOutput=False)
    ident_in = nc.declare_dram_parameter("ident", [128, 128], F32, isOutput=False)
    out_d = nc.declare_dram_parameter("out", [H, W, CO], F32, isOutput=True)

    u2 = nc.dram_tensor("u2", [NT, HP, WPU, CO], BF16)
    offT_d = nc.dram_tensor("offT_d", [2 * NS, 128, 4, 4, 18], F32)

    es = ExitStack()
    _uid = [0]

    def sb(shape, dt):
        _uid[0] += 1
        return es.enter_context(nc.sbuf_tensor(f"sb{_uid[0]}", shape, dt))

    def psum(shape):
        _uid[0] += 1
        return es.enter_context(nc.psum_tensor(f"ps{_uid[0]}", shape, F32))

    def sem(name):
        return es.enter_context(nc.semaphore(name))

    xs = sb([C, 10, 514], BF16)
    w_off_sb = sb([C, NT, 18], BF16)
    b_off_sb = sb([18, 1], F32)
    wd_sb = sb([C, NT * CO], BF16)
    bdef_sb = sb([128, CO], F32)
    ident_sb = sb([128, 128], F32)
    zeros_sb = sb([128, 1024], BF16)
    off_sb = sb([18, 4, W], F32)
    oT_stg = sb([128, 2, 4, 4, 18], F32)
    row_stg = sb([128, 2, 4, NT * CO], BF16)
    offT2 = sb([128, 2, 8, 4, 18], F32)
    ut = sb([128, 2, 14, 12, CO], BF16)
    utp = sb([128, 2, CO, 14, 12], BF16)
    dyc = sb([128, NT, 8, 4], F32)
    dxc = sb([128, NT, 8, 4], F32)
    cy2 = sb([128, NT, 2, 8, 4], F32)   # [.., 0] = (dy-1)+, [.., 1] = (-dy-1)+
    cx2 = sb([128, NT, 2, 8, 4], F32)
    tsc = sb([128, NT, 8, 4], F32)
    tsc2 = sb([128, NT, 8, 4], F32)
    wye = sb([128, NT, 3, 8, 4], F32)
    wxe = sb([128, NT, 3, 8, 4], F32)
    w2m = sb([128, NT, 3, 3, 8, 4], BF16)
    w2ry = sb([128, NT, 2, 3, 8, 4], BF16)
    w2rx = sb([128, NT, 3, 2, 8, 4], BF16)
    accbf = sb([128, 2, CO, 8, 4], BF16)
    mtmp = sb([128, 2, CO, 8, 4], BF16)
    acc = sb([128, CO, 8, 4], F32)
    accT = sb([128, 2, 8, 4, CO], F32)

    off_ps = [psum([18, W]) for _ in range(2)]
    uT_ps8 = [psum([128, 512]) for _ in range(2)]
    uT_ps1 = [psum([128, 512]) for _ in range(2)]   # only [:, 0:64] used
    offT_ps = [psum([128, 4, 4, 18]) for _ in range(2)]

    s_in = sem("s_in"); s_ms = sem("s_ms"); s_x = sem("s_x"); s_z = sem("s_z")
    s_pe = sem("s_pe"); s_vo = sem("s_vo"); s_pt = sem("s_pt"); s_vt = sem("s_vt")
    s_ot = sem("s_ot"); s_pu = sem("s_pu"); s_ac = sem("s_ac"); s_ur = sem("s_ur")
    s_o2 = sem("s_o2"); s_w = sem("s_w"); s_ut = sem("s_ut"); s_up = sem("s_up")
    s_m = sem("s_m"); s_fl = sem("s_fl"); s_ao = sem("s_ao"); s_o = sem("s_o")

    N_IN = 5
    NZ = NT * 6                      # border-zero DMA count

    TAPS = [(t, t // 3, t % 3) for t in range(NT)]

    def corners(t, i, j):
        """(w2 slice, a, b) triples for tap t; rare terms exact for |d|<2."""
        out = []
        for ry in (-1, 0, 1):
            for rx in (-1, 0, 1):
                out.append((w2m[:, t, ry + 1, rx + 1, :, :], (i - 1) + ry, (j - 1) + rx))
        for idx, ry in ((0, 2), (1, -2)):
            for rx in (-1, 0, 1):
                out.append((w2ry[:, t, idx, rx + 1, :, :], (i - 1) + ry, (j - 1) + rx))
        for ry in (-1, 0, 1):
            for idx, rx in ((0, 2), (1, -2)):
                out.append((w2rx[:, t, ry + 1, idx, :, :], (i - 1) + ry, (j - 1) + rx))
        return out

    with nc.Block() as block:

        # ================= SYNC: all loads + stores =================
        @block.sync
        def _(e):
            e.dma_start(w_off_sb[:], w_offT_in[:]).then_inc(s_in, 16)
            e.dma_start(b_off_sb[:], b_off_in[:]).then_inc(s_in, 16)
            e.dma_start(wd_sb[:], wd_in[:]).then_inc(s_in, 16)
            e.dma_start(bdef_sb[:], bdef_in[:]).then_inc(s_in, 16)
            e.dma_start(ident_sb[:], ident_in[:]).then_inc(s_in, 16)

            def stage1(k):
                if k >= 1:
                    e.wait_ge(s_pu, 32 * k)
                r0 = 8 * k - 1
                lo, hi = max(r0, 0), min(r0 + 10, H)
                e.dma_start(xs[:, lo - r0:hi - r0, 1:513], x_in[:, lo:hi, :]
                            ).then_inc(s_x, 16)
                if lo > r0:
                    e.dma_start(xs[:, 0, 1:513], zeros_sb[0:C, 0:512]
                                ).then_inc(s_x, 16)
                if hi - r0 < 10:
                    e.dma_start(xs[:, hi - r0, 1:513], zeros_sb[0:C, 0:512]
                                ).then_inc(s_x, 16)
                for r in range(8):
                    y = 8 * k + r
                    e.wait_ge(s_ac, 4 * y + 4)
                    for b in range(4):
                        dst = u2[:, 3 + y, 4 + 128 * b:4 + 128 * (b + 1), :] \
                            .transpose([1, 0, 2])
                        src = row_stg[:, y % 2, b, :] \
                            .rearrange("p (t c) -> p t c", t=NT)
                        e.dma_start(dst, src).then_inc(s_ur, 16)
                for bt in range(2):
                    gb = 2 * k + bt
                    e.wait_ge(s_vt, gb + 1)
                    e.dma_start(offT_d[gb], oT_stg[:, gb % 2, :, :, :]
                                ).then_inc(s_ot, 16)

            def stage2_load(s):
                # off tile (after stage-1 strip s; offT2 slot free after
                # weights of strip s-2 consumed it)
                if s >= 2:
                    e.wait_ge(s_w, s - 1)
                e.wait_ge(s_ot, (2 * s + 2) * 16)
                dst = offT2[:, s % 2, :, :, :].rearrange("p y q o -> p (y q o)") \
                    .rearrange("p (g f) -> p g f", g=2)
                e.dma_start(dst, offT_d[2 * s:2 * s + 2].rearrange("g p r q o -> p g (r q o)")
                            ).then_inc(s_o2, 16)
                # u tiles, tap-serial, double buffered
                e.wait_ge(s_ur, min(8 * (s + 2), H) * 64)
                e.wait_ge(s_z, NZ * 16)
                for t in range(NT):
                    seq = s * NT + t
                    # ut[t%2] free: last same-slot user is permute seq-1
                    # when t==0 (tap 8 of the previous strip), else seq-2.
                    need = seq if t == 0 else seq - 1
                    if need >= 1:
                        e.wait_ge(s_up, need)
                    for g in range(3):
                        src = u2[t, 8 * s:8 * s + 14, 4 * g:4 * g + 512, :] \
                            .rearrange("y (p q) c -> p y q c", p=128)
                        e.dma_start(ut[:, t % 2, :, 4 * g:4 * g + 4, :], src
                                    ).then_inc(s_ut, 16)

            def stage2_store(s):
                e.wait_ge(s_ao, s + 1)
                dst = out_d[8 * s:8 * s + 8] \
                    .rearrange("y (p b) c -> y p b c", p=128) \
                    .transpose([1, 0, 2, 3])
                e.dma_start(dst, accT[:, s % 2]).then_inc(s_o, 16)

            e.wait_ge(s_ms, 1)
            for k in range(NS):
                stage1(k)
                if k >= 2:
                    stage2_load(k - 2)
                if k >= 3:
                    stage2_store(k - 3)
            for s in (NS - 2, NS - 1):
                stage2_load(s)
            for s in (NS - 3, NS - 2, NS - 1):
                stage2_store(s)
            e.wait_ge(s_o, NS * 16)

        # ================= ACT: zero borders, psum copies, permutes =====
        @block.scalar
        def _(e):
            e.wait_ge(s_ms, 1)
            for t in range(NT):
                top = u2[t, 0:3, :, :].rearrange("y x c -> (y x c)") \
                    .rearrange("(p f) -> p f", p=128)
                e.dma_start(top, zeros_sb[:, 0:3 * WPU * CO // 128]).then_inc(s_z, 16)
                bot = u2[t, HP - 3:HP, :, :].rearrange("y x c -> (y x c)") \
                    .rearrange("(p f) -> p f", p=128)
                e.dma_start(bot, zeros_sb[:, 0:3 * WPU * CO // 128]).then_inc(s_z, 16)
                for x0 in (0, WPU - 4):
                    sl = u2[t, :, x0:x0 + 4, :].rearrange("y x c -> y (x c)")
                    e.dma_start(sl[0:128, :], zeros_sb[:, 0:256]).then_inc(s_z, 16)
                    e.dma_start(sl[128:HP, :], zeros_sb[0:HP - 128, 0:256]
                                ).then_inc(s_z, 16)

            def stage1(k):
                for r in range(8):
                    y = 8 * k + r
                    if y >= 2:
                        e.wait_ge(s_ur, (y - 1) * 64)
                    for b in range(4):
                        g = 4 * y + b
                        pb = g % 2
                        e.wait_ge(s_pu, g + 1)
                        nc.scalar.activation(row_stg[:, y % 2, b, 0:512],
                                             uT_ps8[pb][:], ACTF.Copy)
                        nc.scalar.activation(row_stg[:, y % 2, b, 512:576],
                                             uT_ps1[pb][:, 0:64], ACTF.Copy
                                             ).then_inc(s_ac, 1)

            def stage2(s):
                for t in range(NT):
                    seq = s * NT + t
                    e.wait_ge(s_ut, (seq + 1) * 48)
                    # utp[t%2] free: last same-slot reader is MAC seq-1
                    # when t==0, else seq-2.
                    need = seq if t == 0 else seq - 1
                    if need >= 1:
                        e.wait_ge(s_m, need)
                    src = ut[:, t % 2].transpose([0, 3, 1, 2])
                    nc.scalar.activation(utp[:, t % 2], src, ACTF.Copy
                                         ).then_inc(s_up, 1)
                e.wait_ge(s_fl, NT * (s + 1))
                if s >= 2:
                    e.wait_ge(s_o, (s - 1) * 16)
                nc.scalar.activation(accT[:, s % 2], acc[:].transpose([0, 2, 3, 1]),
                                     ACTF.Copy).then_inc(s_ao, 1)

            for k in range(NS):
                stage1(k)
                if k >= 2:
                    stage2(k - 2)
            for s in (NS - 2, NS - 1):
                stage2(s)

        # ================= PE =================
        @block.tensor
        def _(e):
            e.wait_ge(s_in, N_IN * 16)
            sx_cum = 0
            for k in range(NS):
                sx_cum += 16 * (1 + (k == 0) + (k == NS - 1))
                e.wait_ge(s_x, sx_cum)

                def off_row(r):
                    gr = 8 * k + r
                    pb = gr % 2
                    if gr >= 2:
                        e.wait_ge(s_vo, gr - 1)
                    mm = None
                    for t in range(NT):
                        i, j = t // 3, t % 3
                        mm = nc.tensor.matmul(off_ps[pb][:], w_off_sb[:, t, :],
                                              xs[:, r + i, j:j + 512],
                                              start=(t == 0), stop=(t == NT - 1))
                    mm.then_inc(s_pe, 1)

                def tr_batch(bt):
                    gb = 2 * k + bt
                    pb = gb % 2
                    e.wait_ge(s_vo, 8 * k + 4 * (bt + 1))
                    if gb >= 2:
                        e.wait_ge(s_vt, gb - 1)
                    mm = None
                    for rr in range(4):
                        for s4 in range(4):
                            mm = nc.tensor.transpose(
                                offT_ps[pb][:, rr, s4, :],
                                off_sb[:, (4 * bt + rr) % 4, s4::4],
                                ident_sb[0:18, 0:18])
                    mm.then_inc(s_pt, 1)

                for r in range(4):
                    off_row(r)
                tr_batch(0)
                for r in range(4, 8):
                    off_row(r)
                tr_batch(1)
                for lc in range(32):
                    g = 32 * k + lc
                    pb = g % 2
                    r, b = lc // 4, lc % 4
                    if g >= 2:
                        e.wait_ge(s_ac, g - 1)
                    lhsT = xs[:, r + 1, 1 + b * 128:1 + b * 128 + 128]
                    nc.tensor.matmul(uT_ps8[pb][:], lhsT, wd_sb[:, 0:512],
                                     start=True, stop=True)
                    nc.tensor.matmul(uT_ps1[pb][:, 0:64], lhsT, wd_sb[:, 512:576],
                                     start=True, stop=True).then_inc(s_pu, 1)

        # ================= DVE =================
        @block.vector
        def _(e):
            nc.vector.memset(xs[:], 0.0)
            nc.vector.memset(zeros_sb[:], 0.0)
            e.drain()
            e.sem_inc(s_ms, 1)
            e.wait_ge(s_in, N_IN * 16)

            def stage1(k):
                for r in range(8):
                    gr = 8 * k + r
                    pb = gr % 2
                    e.wait_ge(s_pe, gr + 1)
                    if gr >= 4:
                        e.wait_ge(s_pt, (gr - 4) // 4 + 1)
                    nc.vector.tensor_scalar(off_sb[:, r % 4, :], off_ps[pb][:],
                                            b_off_sb[:], 0.0, OP.add, OP.add
                                            ).then_inc(s_vo, 1)
                for bt in range(2):
                    gb = 2 * k + bt
                    pb = gb % 2
                    e.wait_ge(s_pt, gb + 1)
                    if gb >= 2:
                        e.wait_ge(s_ot, gb * 16)
                    nc.vector.tensor_copy(oT_stg[:, gb % 2], offT_ps[pb][:]
                                          ).then_inc(s_vt, 1)

            def weights(s):
                e.wait_ge(s_o2, 16 * (s + 1))
                offv = offT2[:, s % 2].rearrange("p y q (t two) -> p y q t two", two=2)
                dyv = offv[:, :, :, :, 0].transpose([0, 3, 1, 2])  # [128, 9, 8, 4]
                dxv = offv[:, :, :, :, 1].transpose([0, 3, 1, 2])
                ts = nc.vector.tensor_scalar
                tt = nc.vector.tensor_tensor
                for dv, dc, c2, we in ((dyv, dyc, cy2, wye), (dxv, dxc, cx2, wxe)):
                    ts(dc[:], dv, 1.0, -1.0, OP.min, OP.max)
                    ts(c2[:, :, 0], dv, -1.0, 0.0, OP.add, OP.max)
                    ts(tsc[:], dv, 1.0, 0.0, OP.add, OP.min)
                    ts(c2[:, :, 1], tsc[:], -1.0, 0.0, OP.mult, OP.max)
                    for r in (-1, 0, 1):
                        # tri(dc - r) = max(0, min(1-(dc-r), 1+(dc-r)))
                        ts(tsc[:], dc[:], -1.0, 1.0 + r, OP.mult, OP.add)
                        ts(tsc2[:], dc[:], 1.0 - r, 0.0, OP.add, OP.add)
                        tt(tsc[:], tsc[:], tsc2[:], OP.min)
                        ts(we[:, :, r + 1], tsc[:], 0.0, 0.0, OP.max, OP.add)
                    tt(we[:, :, 2], we[:, :, 2], c2[:, :, 0], OP.subtract)
                    tt(we[:, :, 0], we[:, :, 0], c2[:, :, 1], OP.subtract)
                tt(w2m[:], wye[:].unsqueeze(3).broadcast_to([128, NT, 3, 3, 8, 4]),
                   wxe[:].unsqueeze(2).broadcast_to([128, NT, 3, 3, 8, 4]), OP.mult)
                tt(w2ry[:], cy2[:].unsqueeze(3).broadcast_to([128, NT, 2, 3, 8, 4]),
                   wxe[:].unsqueeze(2).broadcast_to([128, NT, 2, 3, 8, 4]), OP.mult)
                tt(w2rx[:], wye[:].unsqueeze(3).broadcast_to([128, NT, 3, 2, 8, 4]),
                   cx2[:].unsqueeze(2).broadcast_to([128, NT, 3, 2, 8, 4]), OP.mult)
                e.sem_inc(s_w, 1)

            def mac(s):
                for t, i, j in TAPS:
                    seq = s * NT + t
                    tp = t % 2
                    e.wait_ge(s_up, seq + 1)
                    # accbf[tp] free: last same-slot reader is flush seq-1
                    # when t==0, else seq-2.
                    need = seq if t == 0 else seq - 1
                    if need >= 1:
                        e.wait_ge(s_fl, need)
                    last = None
                    for kk, (wsl, a, b) in enumerate(corners(t, i, j)):
                        usl = utp[:, tp, :, 3 + a:11 + a, 4 + b:8 + b]
                        wv = wsl.unsqueeze(1).broadcast_to([128, CO, 8, 4])
                        if kk == 0:
                            last = nc.vector.tensor_tensor(
                                accbf[:, tp], usl, wv, OP.mult)
                        else:
                            nc.vector.tensor_tensor(mtmp[:, kk % 2], usl, wv, OP.mult)
                            last = nc.vector.scalar_tensor_tensor(
                                accbf[:, tp], mtmp[:, kk % 2], 1.0, accbf[:, tp],
                                OP.mult, OP.add)
                    last.then_inc(s_m, 1)

            for k in range(NS):
                stage1(k)
                if k >= 2:
                    weights(k - 2)
                    mac(k - 2)
            for s in (NS - 2, NS - 1):
                weights(s)
                mac(s)

        # ================= POOL: per-tap flush into f32 acc =============
        @block.gpsimd
        def _(e):
            bdefv = bdef_sb[:].unsqueeze(2).unsqueeze(3).broadcast_to([128, CO, 8, 4])
            e.wait_ge(s_in, N_IN * 16)
            for s in range(NS):
                for t in range(NT):
                    seq = s * NT + t
                    e.wait_ge(s_m, seq + 1)
                    if t == 0:
                        if s >= 1:
                            e.wait_ge(s_ao, s)
                        nc.gpsimd.tensor_tensor(acc[:], bdefv, accbf[:, 0], OP.add
                                                ).then_inc(s_fl, 1)
                    else:
                        nc.gpsimd.tensor_tensor(
                            acc[:], acc[:], accbf[:, t % 2], OP.add
                        ).then_inc(s_fl, 1)

    es.close()
    nc.compile()
    return nc


def host_inputs(x_n, w_off, b_off, w_def, b_def):
    """Per-core input map for one sample (np arrays as in setup_inputs)."""
    import ml_dtypes
    w_offT = np.ascontiguousarray(
        w_off.reshape(18, C, NT).transpose(1, 2, 0)).astype(ml_dtypes.bfloat16)
    wd = np.ascontiguousarray(
        w_def.reshape(CO, C, NT).transpose(1, 2, 0).reshape(C, NT * CO)
    ).astype(ml_dtypes.bfloat16)
    return {
        "x": np.ascontiguousarray(x_n).astype(ml_dtypes.bfloat16),
        "w_offT": w_offT,
        "b_off": np.ascontiguousarray(b_off.reshape(18, 1), np.float32),
        "wd_all": wd,
        "bdef": np.ascontiguousarray(np.tile(b_def[None, :], (128, 1)), np.float32),
        "ident": np.eye(128, dtype=np.float32),
    }


_CACHE = {}


def _get_nc():
    if "nc" not in _CACHE:
        _CACHE["nc"] = build(H=128)
    return _CACHE["nc"]


def kernel(x, w_off, b_off, w_def, b_def):
    """Full-input DeformConv2d on 8 NeuronCores (one sample per core)."""
    from concourse.bass_utils import run_bass_kernel_spmd

    x = np.asarray(x, np.float32)
    w_off = np.asarray(w_off, np.float32)
    b_off = np.asarray(b_off, np.float32)
    w_def = np.asarray(w_def, np.float32)
    b_def = np.asarray(b_def, np.float32)
    N, Cc, H, Wc = x.shape
    assert (N, Cc, H, Wc) == (8, 64, 128, 512)

    nc = _get_nc()
    shared = host_inputs(x[0], w_off, b_off, w_def, b_def)
    in_maps = []
    for n in range(N):
        m = dict(shared)
        m["x"] = np.ascontiguousarray(x[n]).astype(shared["x"].dtype)
        in_maps.append(m)
    res = run_bass_kernel_spmd(nc, in_maps, list(range(N)))
    out = np.empty((N, CO, H, Wc), np.float32)
    for n in range(N):
        o = res.results[n]["out"]          # [H, W, CO]
        out[n] = o.transpose(2, 0, 1)
    return out
